# revision 13
# baseline (speedup 1.0000x reference)
"""Trainium2 Bass kernel for causal multi-head attention (B=4, T=2048, D=1024, H=16).

Sharding: tensor-parallel over heads. Each of the 8 NeuronCores owns 2 heads
(128 of the 1024 attention features): it computes Q/K/V projections for its
head-slice over all tokens, runs causal attention, then a bf16 AllToAll
re-shards the attention output from head-sharded to token-sharded so each core
computes the final FC layer (full W_fc) for its 256-token block per batch.

Fast paths used here:
- all projection matmuls run in bf16 (1 cycle/row at any moving-dim size);
  score matmuls in f32r (TF32-like, full rate at moving dim >= 256).
- biases are fused into the PSUM->SBUF evacuation copies (tensor_scalar_add
  with a per-partition scalar), no bias matmuls.
- V is projected feature-major (weights stationary) and transposed to
  token-major with PE transposes, avoiding 4x as many LDWEIGHTS.
- scores are computed transposed (S^T = K Q^T); the softmax denominator comes
  from a ones-column appended to V; its reciprocal is computed directly from
  PSUM with reciprocal_approx_fast and broadcast across partitions with a
  K=1 selector matmul.
- diagonal-block score matmuls and exps only cover the non-fully-masked
  columns; dedicated pre-zeroed exp-output tiles keep the PV accumulation
  correct without masking work.
- the emission order software-pipelines: proj(b+1) and FC(b-1) overlap the
  AllToAll of batch b.
"""
import sys

for _p in ("/opt/trn_rl_repo",):
    if _p not in sys.path:
        sys.path.insert(0, _p)

import numpy as np

import concourse.bass as bass
import concourse.mybir as mybir
import concourse.tile as tile
from concourse import bacc
from concourse.bass_utils import run_bass_kernel_spmd

f32 = mybir.dt.float32
f32r = mybir.dt.float32r
bf16 = mybir.dt.bfloat16
EXP = mybir.ActivationFunctionType.Exp

B, T, D, H, HD = 4, 2048, 1024, 16, 64
NCORES = 8
HPC = H // NCORES          # heads per core = 2
BT = B * T                 # 8192
CH = 512                   # token chunk (matmul moving dim)
NCH_B = T // CH            # 4 projection chunks per batch
QC = T // CH               # 4 query chunks per batch
TPB = T // NCORES          # 256 tokens per core per batch
ROWS = B * TPB             # 1024 output token rows per core
SCALE = 1.0 / 8.0          # 1/sqrt(HD)

_CACHE = {}


def _build(sim=False, no_collective=False, debug_out=False):
    nc = bacc.Bacc("TRN2", target_bir_lowering=False, debug=False,
                   num_devices=1 if sim else NCORES)

    xT = nc.dram_tensor("xT", [D, BT], bf16, kind="ExternalInput").ap()
    wqkv = nc.dram_tensor("wqkv", [D, 3 * 128], bf16, kind="ExternalInput").ap()
    wfc_d = nc.dram_tensor("wfc", [D, D], bf16, kind="ExternalInput").ap()
    bq_d = nc.dram_tensor("bqc", [128, 3], f32, kind="ExternalInput").ap()
    bfc_d = nc.dram_tensor("bfcc", [128, 8], f32, kind="ExternalInput").ap()
    ht_d = nc.dram_tensor("ht", [128, 128], bf16, kind="ExternalInput").ap()
    ones_d = nc.dram_tensor("ones1", [65, 64], f32, kind="ExternalInput").ap()
    id_d = nc.dram_tensor("ident", [128, 128], f32, kind="ExternalInput").ap()
    outT = nc.dram_tensor("outT", [D, ROWS], f32, kind="ExternalOutput").ap()
    if debug_out:
        dq = nc.dram_tensor("dq", [128, T], f32, kind="ExternalOutput").ap()
        dk = nc.dram_tensor("dk", [128, T], f32, kind="ExternalOutput").ap()
        dv = nc.dram_tensor("dv", [128, 16 * 130], bf16, kind="ExternalOutput").ap()
        da0 = nc.dram_tensor("da0", [64, T], bf16, kind="ExternalOutput").ap()
        da1 = nc.dram_tensor("da1", [64, T], bf16, kind="ExternalOutput").ap()
        drc = nc.dram_tensor("drc", [1, 2 * T], f32, kind="ExternalOutput").ap()
        dao = nc.dram_tensor("dao", [NCORES * 128, TPB], bf16, kind="ExternalOutput").ap()

    with tile.TileContext(nc) as tc:
        with tc.tile_pool(name="const", bufs=1) as cst, \
             tc.tile_pool(name="dram", bufs=1, space="DRAM") as dpool, \
             tc.tile_pool(name="work", bufs=1) as wk, \
             tc.tile_pool(name="ps", bufs=1, space="PSUM") as ps:

            # ---- constants ----
            htm = cst.tile([128, 128], bf16)
            nc.sync.dma_start(htm[:], ht_d[:])
            idm = cst.tile([128, 128], f32)
            nc.sync.dma_start(idm[:], id_d[:])
            bqc = cst.tile([128, 3], f32)
            nc.sync.dma_start(bqc[:], bq_d[:])
            bfcc = cst.tile([128, 8], f32)
            nc.sync.dma_start(bfcc[:], bfc_d[:])
            ones1 = cst.tile([65, 64], f32)       # row 64 = 1 (selector lhsT)
            nc.sync.dma_start(ones1[:], ones_d[:])
            # ln(denominator) staging: row 64 = ln(den h0), cols T.. = h1
            lnd = cst.tile([65, 2 * T], f32)
            nc.gpsimd.memset(lnd[0:64, :], 0.0)
            # dedicated exp-output tiles for the 4 diagonal offsets; the
            # columns the restricted exp never writes must stay zero.
            ptd = []
            for d in range(4):
                t = cst.tile([128, 2 * CH], bf16, name=f"ptd{d}")
                nc.gpsimd.memset(t[:], 0.0)
                ptd.append(t)

            # qkv weights: 8 d-tiles of [128, 384] = [q128 | k128 | v128]
            wq = cst.tile([128, 8 * 384], bf16)
            for d in range(8):
                nc.sync.dma_start(wq[:, d * 384:(d + 1) * 384],
                                  wqkv[d * 128:(d + 1) * 128, :])
            # full FC weight: [128 (in-feat within d-tile), d-tile * 1024]
            wfc = cst.tile([128, 8 * D], bf16)
            for d in range(8):
                nc.sync.dma_start(wfc[:, d * D:(d + 1) * D],
                                  wfc_d[d * 128:(d + 1) * 128, :])

            qt = {}
            kt = {}
            vsb = {}
            attn = {}
            a2a_out = {}

            def emit_proj(b):
                t0 = b * T
                qt[b] = wk.tile([128, T], f32r, tag="qt", bufs=2, name=f"qt{b}")
                kt[b] = wk.tile([128, T], f32r, tag="kt", bufs=2, name=f"kt{b}")
                vsb[b] = wk.tile([128, 4 * NCH_B * 130], bf16, tag="vsb",
                                 bufs=2, name=f"vsb{b}")
                v3 = vsb[b].rearrange("p (t c) -> p t c", c=130)
                nc.gpsimd.memset(v3[:, :, 64:65], 1.0)
                nc.gpsimd.memset(v3[:, :, 129:130], 1.0)
                for ch in range(NCH_B):
                    c0 = t0 + ch * CH
                    cs = ch * CH
                    xt = wk.tile([128, 8 * CH], bf16, tag="xt", bufs=2,
                                 name=f"xt{b}_{ch}")
                    xt3 = xt.rearrange("p (d c) -> p d c", d=8)
                    xs3 = xT[:, c0:c0 + CH].rearrange("(d p) c -> p d c", p=128)
                    nc.sync.dma_start(xt3[:], xs3)
                    # Q^T chunk
                    psq = ps.tile([128, CH], f32, tag="mm", bufs=2,
                                  name=f"psq{b}_{ch}")
                    for d in range(8):
                        nc.tensor.matmul(psq[:], wq[:, d * 384:d * 384 + 128],
                                         xt[:, d * CH:(d + 1) * CH],
                                         start=(d == 0), stop=(d == 7))
                    nc.vector.tensor_scalar_add(qt[b][:, cs:cs + CH], psq[:],
                                                bqc[:, 0:1])
                    # K^T chunk
                    psk = ps.tile([128, CH], f32, tag="mm", bufs=2,
                                  name=f"psk{b}_{ch}")
                    for d in range(8):
                        nc.tensor.matmul(psk[:],
                                         wq[:, d * 384 + 128:d * 384 + 256],
                                         xt[:, d * CH:(d + 1) * CH],
                                         start=(d == 0), stop=(d == 7))
                    nc.vector.tensor_scalar_add(kt[b][:, cs:cs + CH], psk[:],
                                                bqc[:, 1:2])
                    # V^T chunk (feature-major), then PE-transpose to
                    # token-major with the ones column interleaved
                    psv = ps.tile([128, CH], f32, tag="mm", bufs=2,
                                  name=f"psv{b}_{ch}")
                    for d in range(8):
                        nc.tensor.matmul(psv[:],
                                         wq[:, d * 384 + 256:d * 384 + 384],
                                         xt[:, d * CH:(d + 1) * CH],
                                         start=(d == 0), stop=(d == 7))
                    vtmp = wk.tile([128, CH], f32, tag="vt", bufs=2,
                                   name=f"vtmp{b}_{ch}")
                    nc.vector.tensor_scalar_add(vtmp[:], psv[:], bqc[:, 2:3])
                    psvt = ps.tile([128, CH], f32, tag="mm", bufs=2,
                                   name=f"psvt{b}_{ch}")
                    for sb in range(4):
                        nc.tensor.transpose(psvt[:, sb * 128:(sb + 1) * 128],
                                            vtmp[:, sb * 128:(sb + 1) * 128],
                                            idm[:])
                    for sb in range(4):
                        base = (ch * 4 + sb) * 130
                        nc.vector.tensor_copy(
                            vsb[b][:, base:base + 64],
                            psvt[:, sb * 128:sb * 128 + 64])
                        nc.vector.tensor_copy(
                            vsb[b][:, base + 65:base + 129],
                            psvt[:, sb * 128 + 64:sb * 128 + 128])

            def emit_attention(b):
                attn[b] = [wk.tile([64, T], bf16, tag=f"attn{h}", bufs=2,
                                   name=f"attn{h}_{b}") for h in range(HPC)]
                for qc in range(QC):
                    qs = qc * CH
                    nkv = 4 * (qc + 1)
                    pv = [ps.tile([65, CH], f32, tag=f"pv{h}", bufs=1,
                                  name=f"pv{h}_{b}_{qc}")
                          for h in range(HPC)]
                    for ki in range(nkv):
                        diag = ki - 4 * qc
                        c0 = 128 * diag if diag >= 0 else 0
                        st = ps.tile([128, 2 * CH], f32, tag="s", bufs=2,
                                     name=f"s_{b}_{qc}_{ki}")
                        st3 = st.rearrange("p (h c) -> p h c", h=2)
                        for h in range(HPC):
                            nc.tensor.matmul(
                                st3[:, h, c0:CH],
                                kt[b][64 * h:64 * h + 64,
                                      ki * 128:(ki + 1) * 128],
                                qt[b][64 * h:64 * h + 64,
                                      qs + c0:qs + CH],
                                start=True, stop=True,
                                tile_position=(64 * h, 0))
                        if diag >= 0:
                            ptx = ptd[diag]
                        else:
                            ptx = wk.tile([128, 2 * CH], bf16, tag="p",
                                          bufs=3, name=f"p_{b}_{qc}_{ki}")
                        pt3 = ptx.rearrange("p (h c) -> p h c", h=2)
                        nc.scalar.activation(pt3[:, :, c0:CH], st3[:, :, c0:CH],
                                             EXP, scale=SCALE)
                        if diag >= 0:
                            for h in range(HPC):
                                nc.gpsimd.tensor_mul(pt3[:, h, c0:c0 + 128],
                                                     pt3[:, h, c0:c0 + 128],
                                                     htm[:])
                        for h in range(HPC):
                            vb = ki * 130 + 65 * h
                            nc.tensor.matmul(pv[h][0:65, :],
                                             vsb[b][:, vb:vb + 65],
                                             ptx[:, h * CH:(h + 1) * CH],
                                             start=(ki == 0),
                                             stop=(ki == nkv - 1))
                    # unnormalized outputs + reciprocal of denominators
                    nc.vector.tensor_copy(attn[b][0][:, qs:qs + CH],
                                          pv[0][0:64, :])
                    nc.vector.tensor_copy(attn[b][1][:, qs:qs + CH],
                                          pv[1][0:64, :])
                    nc.scalar.activation(lnd[64:65, qs:qs + CH],
                                         pv[0][64:65, :],
                                         mybir.ActivationFunctionType.Ln)
                    nc.scalar.activation(lnd[64:65, T + qs:T + qs + CH],
                                         pv[1][64:65, :],
                                         mybir.ActivationFunctionType.Ln)
                # normalize: broadcast reciprocals across partitions via K=1
                # selector matmul, then one in-place multiply per q-chunk
                for qc in range(QC):
                    qs = qc * CH
                    for h in range(HPC):
                        bc = ps.tile([128, CH], f32, tag="mm", bufs=2,
                                     name=f"bc{b}_{qc}_{h}")
                        nc.tensor.matmul(bc[0:64, :], ones1[:],
                                         lnd[0:65, h * T + qs:h * T + qs + CH],
                                         start=True, stop=True)
                        rb = wk.tile([64, CH], bf16, tag="rb", bufs=2,
                                     name=f"rb{b}_{qc}_{h}")
                        nc.scalar.activation(rb[:], bc[0:64, :], EXP,
                                             scale=-1.0)
                        nc.vector.tensor_mul(attn[b][h][:, qs:qs + CH],
                                             attn[b][h][:, qs:qs + CH],
                                             rb[:])
                # re-shard head-sharded -> token-sharded
                a2a_in = dpool.tile([NCORES * 128, TPB], bf16,
                                    name=f"a2ai{b}")
                a2a_out[b] = dpool.tile([NCORES * 128, TPB], bf16,
                                        name=f"a2ao{b}")
                a2a_v = a2a_in.rearrange("(r hp p) c -> p r hp c",
                                         p=64, hp=2)
                for h in range(HPC):
                    nc.sync.dma_start(
                        a2a_v[:, :, h, :],
                        attn[b][h][:].rearrange("p (r c) -> p r c", c=TPB))
                if sim or no_collective:
                    nc.sync.dma_start(a2a_out[b][:], a2a_in[:])
                else:
                    nc.gpsimd.collective_compute(
                        "AllToAll", mybir.AluOpType.bypass,
                        replica_groups=[list(range(NCORES))],
                        ins=[a2a_in.opt()], outs=[a2a_out[b].opt()])

            def emit_fc(b):
                fci = wk.tile([128, 8 * TPB], bf16, tag="fci", bufs=2,
                              name=f"fci{b}")
                nc.sync.dma_start(
                    fci.rearrange("p (d c) -> p d c", d=8),
                    a2a_out[b].rearrange("(d p) c -> p d c", p=128))
                osts = wk.tile([128, 8 * TPB], f32, tag="ost", bufs=2,
                               name=f"ost{b}")
                for m in range(8):
                    pfc = ps.tile([128, CH], f32, tag="mm", bufs=2,
                                  name=f"pfc{b}_{m}")
                    for d in range(8):
                        nc.tensor.matmul(
                            pfc[:, 0:TPB],
                            wfc[:, d * D + m * 128:d * D + (m + 1) * 128],
                            fci[:, d * TPB:(d + 1) * TPB],
                            start=(d == 0), stop=(d == 7))
                    nc.vector.tensor_scalar_add(
                        osts[:, m * TPB:(m + 1) * TPB], pfc[:, 0:TPB],
                        bfcc[:, m:m + 1])
                nc.sync.dma_start(
                    outT[:, b * TPB:(b + 1) * TPB].rearrange(
                        "(m p) c -> p m c", p=128),
                    osts[:].rearrange("p (m c) -> p m c", c=TPB))

            emit_proj(0)
            if debug_out:
                nc.sync.dma_start(dq[:], qt[0][:].bitcast(f32))
                nc.sync.dma_start(dk[:], kt[0][:].bitcast(f32))
                nc.sync.dma_start(dv[:], vsb[0][:])
            for b in range(B):
                emit_attention(b)
                if debug_out and b == 0:
                    nc.sync.dma_start(da0[:], attn[0][0][:])
                    nc.sync.dma_start(da1[:], attn[0][1][:])
                    nc.sync.dma_start(drc[:], lnd[64:65, :])
                    nc.sync.dma_start(dao[:], a2a_out[0][:])
                if b + 1 < B:
                    emit_proj(b + 1)
                if b >= 1:
                    emit_fc(b - 1)
            emit_fc(B - 1)

    nc.compile()
    return nc


def _host_inputs(x, W_qkv, b_qkv, W_fc, b_fc):
    import ml_dtypes
    x = np.asarray(x, dtype=np.float32)
    W_qkv = np.asarray(W_qkv, dtype=np.float32)
    b_qkv = np.asarray(b_qkv, dtype=np.float32)
    W_fc = np.asarray(W_fc, dtype=np.float32)
    b_fc = np.asarray(b_fc, dtype=np.float32)

    xT = np.ascontiguousarray(x.reshape(BT, D).T).astype(ml_dtypes.bfloat16)
    wfc_b = np.ascontiguousarray(W_fc).astype(ml_dtypes.bfloat16)
    bfcc = np.ascontiguousarray(b_fc.reshape(8, 128).T)   # [128, 8]
    ht = (np.arange(128)[None, :] >= np.arange(128)[:, None]).astype(
        ml_dtypes.bfloat16)                               # keep iff c >= r
    ident = np.eye(128, dtype=np.float32)
    ones1 = np.zeros((65, 64), dtype=np.float32)
    ones1[64, :] = 1.0
    in_maps = []
    for c in range(NCORES):
        f0 = c * (HPC * HD)  # 128*c
        wq_c = np.ascontiguousarray(np.concatenate(
            [W_qkv[:, p * D + f0: p * D + f0 + 128] for p in range(3)],
            axis=1).astype(ml_dtypes.bfloat16))
        bq_c = np.ascontiguousarray(np.stack(
            [b_qkv[p * D + f0: p * D + f0 + 128] for p in range(3)],
            axis=1))                                      # [128, 3]
        in_maps.append({
            "xT": xT, "wqkv": wq_c, "wfc": wfc_b, "bqc": bq_c, "bfcc": bfcc,
            "ht": ht, "ident": ident, "ones1": ones1,
        })
    return in_maps


def _get_nc():
    if "nc" not in _CACHE:
        _CACHE["nc"] = _build()
    return _CACHE["nc"]


def _assemble(results):
    out = np.empty((BT, D), dtype=np.float32)
    for c in range(NCORES):
        o = results[c]["outT"]                  # [1024 feat, 1024 tok]
        for b in range(B):
            r0 = b * T + c * TPB
            out[r0:r0 + TPB, :] = o[:, b * TPB:(b + 1) * TPB].T
    return out.reshape(B, T, D)


def kernel(x, W_qkv, b_qkv, W_fc, b_fc):
    nc = _get_nc()
    in_maps = _host_inputs(x, W_qkv, b_qkv, W_fc, b_fc)
    res = run_bass_kernel_spmd(nc, in_maps, list(range(NCORES)))
    return _assemble(res.results)


# revision 14
# speedup vs baseline: 2.2132x; 2.2132x over previous
"""Trainium2 Bass kernel for causal multi-head attention (B=4, T=2048, D=1024, H=16).

Sharding: tensor-parallel over heads. Each of the 8 NeuronCores owns 2 heads
(128 of the 1024 attention features): it computes Q/K/V projections for its
head-slice over all tokens, runs causal attention, then a bf16 AllToAll
re-shards the attention output from head-sharded to token-sharded so each core
computes the final FC layer (full W_fc) for its 256-token block per batch.

Fast paths used here:
- all projection matmuls run in bf16 (1 cycle/row at any moving-dim size);
  score matmuls in f32r (TF32-like, full rate at moving dim >= 256).
- biases are fused into the PSUM->SBUF evacuation copies (tensor_scalar_add
  with a per-partition scalar), no bias matmuls.
- V is projected feature-major (weights stationary) and transposed to
  token-major with PE transposes, avoiding 4x as many LDWEIGHTS.
- scores are computed transposed (S^T = K Q^T); the softmax denominator comes
  from a ones-column appended to V; its reciprocal is computed directly from
  PSUM with reciprocal_approx_fast and broadcast across partitions with a
  K=1 selector matmul.
- diagonal-block score matmuls and exps only cover the non-fully-masked
  columns; dedicated pre-zeroed exp-output tiles keep the PV accumulation
  correct without masking work.
- the emission order software-pipelines: proj(b+1) and FC(b-1) overlap the
  AllToAll of batch b.
"""
import sys

for _p in ("/opt/trn_rl_repo",):
    if _p not in sys.path:
        sys.path.insert(0, _p)

import numpy as np

import concourse.bass as bass
import concourse.mybir as mybir
import concourse.tile as tile
from concourse import bacc
from concourse.bass_utils import run_bass_kernel_spmd

f32 = mybir.dt.float32
f32r = mybir.dt.float32r
bf16 = mybir.dt.bfloat16
EXP = mybir.ActivationFunctionType.Exp

B, T, D, H, HD = 4, 2048, 1024, 16, 64
NCORES = 8
HPC = H // NCORES          # heads per core = 2
BT = B * T                 # 8192
CH = 512                   # token chunk (matmul moving dim)
NCH_B = T // CH            # 4 projection chunks per batch
QC = T // CH               # 4 query chunks per batch
TPB = T // NCORES          # 256 tokens per core per batch
ROWS = B * TPB             # 1024 output token rows per core
SCALE = 1.0 / 8.0          # 1/sqrt(HD)

_CACHE = {}


def _build(sim=False, no_collective=False, debug_out=False):
    nc = bacc.Bacc("TRN2", target_bir_lowering=False, debug=False,
                   num_devices=1 if sim else NCORES)

    xT = nc.dram_tensor("xT", [D, BT], bf16, kind="ExternalInput").ap()
    wqkv = nc.dram_tensor("wqkv", [D, 3 * 128], bf16, kind="ExternalInput").ap()
    wfc_d = nc.dram_tensor("wfc", [D, D], bf16, kind="ExternalInput").ap()
    bq_d = nc.dram_tensor("bqc", [128, 3], f32, kind="ExternalInput").ap()
    bfc_d = nc.dram_tensor("bfcc", [128, 8], f32, kind="ExternalInput").ap()
    ht_d = nc.dram_tensor("ht", [128, 128], bf16, kind="ExternalInput").ap()
    ones_d = nc.dram_tensor("ones1", [65, 64], f32, kind="ExternalInput").ap()
    id_d = nc.dram_tensor("ident", [128, 128], f32, kind="ExternalInput").ap()
    outT = nc.dram_tensor("outT", [D, ROWS], f32, kind="ExternalOutput").ap()
    if debug_out:
        dq = nc.dram_tensor("dq", [128, T], f32, kind="ExternalOutput").ap()
        dk = nc.dram_tensor("dk", [128, T], f32, kind="ExternalOutput").ap()
        dv = nc.dram_tensor("dv", [128, 16 * 130], bf16, kind="ExternalOutput").ap()
        da0 = nc.dram_tensor("da0", [64, T], bf16, kind="ExternalOutput").ap()
        da1 = nc.dram_tensor("da1", [64, T], bf16, kind="ExternalOutput").ap()
        drc = nc.dram_tensor("drc", [1, 2 * T], f32, kind="ExternalOutput").ap()
        dao = nc.dram_tensor("dao", [NCORES * 128, TPB], bf16, kind="ExternalOutput").ap()

    with tile.TileContext(nc) as tc:
        with tc.tile_pool(name="const", bufs=1) as cst, \
             tc.tile_pool(name="dram", bufs=1, space="DRAM") as dpool, \
             tc.tile_pool(name="work", bufs=1) as wk, \
             tc.tile_pool(name="ps", bufs=1, space="PSUM") as ps:

            # ---- constants ----
            htm = cst.tile([128, 128], bf16)
            nc.sync.dma_start(htm[:], ht_d[:])
            idm = cst.tile([128, 128], f32)
            nc.sync.dma_start(idm[:], id_d[:])
            bqc = cst.tile([128, 3], f32)
            nc.sync.dma_start(bqc[:], bq_d[:])
            bfcc = cst.tile([128, 8], f32)
            nc.sync.dma_start(bfcc[:], bfc_d[:])
            ones1 = cst.tile([65, 64], f32)       # row 64 = 1 (selector lhsT)
            nc.sync.dma_start(ones1[:], ones_d[:])
            # ln(denominator) staging: row 64 = ln(den h0), cols T.. = h1
            lnd = cst.tile([65, 2 * T], f32)
            nc.gpsimd.memset(lnd[0:64, :], 0.0)
            den_s = cst.tile([65, 2 * T], f32)
            # dedicated exp-output tiles for the 4 diagonal offsets; the
            # columns the restricted exp never writes must stay zero.
            ptd = []
            for d in range(4):
                t = cst.tile([128, 2 * CH], bf16, name=f"ptd{d}")
                nc.gpsimd.memset(t[:], 0.0)
                ptd.append(t)

            # qkv weights: 8 d-tiles of [128, 384] = [q128 | k128 | v128]
            wq = cst.tile([128, 8 * 384], bf16)
            for d in range(8):
                nc.sync.dma_start(wq[:, d * 384:(d + 1) * 384],
                                  wqkv[d * 128:(d + 1) * 128, :])
            # full FC weight: [128 (in-feat within d-tile), d-tile * 1024]
            wfc = cst.tile([128, 8 * D], bf16)

            qt = {}
            kt = {}
            vsb = {}
            attn = {}
            a2a_out = {}

            def emit_proj(b):
                t0 = b * T
                qt[b] = wk.tile([128, T], f32r, tag="qt", bufs=2, name=f"qt{b}")
                kt[b] = wk.tile([128, T], f32r, tag="kt", bufs=2, name=f"kt{b}")
                vsb[b] = wk.tile([128, 4 * NCH_B * 130], bf16, tag="vsb",
                                 bufs=2, name=f"vsb{b}")
                v3 = vsb[b].rearrange("p (t c) -> p t c", c=130)
                nc.gpsimd.memset(v3[:, :, 64:65], 1.0)
                nc.gpsimd.memset(v3[:, :, 129:130], 1.0)
                for ch in range(NCH_B):
                    c0 = t0 + ch * CH
                    cs = ch * CH
                    xt = wk.tile([128, 8 * CH], bf16, tag="xt", bufs=2,
                                 name=f"xt{b}_{ch}")
                    xt3 = xt.rearrange("p (d c) -> p d c", d=8)
                    xs3 = xT[:, c0:c0 + CH].rearrange("(d p) c -> p d c", p=128)
                    nc.sync.dma_start(xt3[:], xs3)
                    # Q^T chunk
                    psq = ps.tile([128, CH], f32, tag="mm", bufs=2,
                                  name=f"psq{b}_{ch}")
                    for d in range(8):
                        nc.tensor.matmul(psq[:], wq[:, d * 384:d * 384 + 128],
                                         xt[:, d * CH:(d + 1) * CH],
                                         start=(d == 0), stop=(d == 7))
                    nc.vector.tensor_scalar_add(qt[b][:, cs:cs + CH], psq[:],
                                                bqc[:, 0:1])
                    # K^T chunk
                    psk = ps.tile([128, CH], f32, tag="mm", bufs=2,
                                  name=f"psk{b}_{ch}")
                    for d in range(8):
                        nc.tensor.matmul(psk[:],
                                         wq[:, d * 384 + 128:d * 384 + 256],
                                         xt[:, d * CH:(d + 1) * CH],
                                         start=(d == 0), stop=(d == 7))
                    nc.vector.tensor_scalar_add(kt[b][:, cs:cs + CH], psk[:],
                                                bqc[:, 1:2])
                    # V^T chunk (feature-major), then PE-transpose to
                    # token-major with the ones column interleaved
                    psv = ps.tile([128, CH], f32, tag="mm", bufs=2,
                                  name=f"psv{b}_{ch}")
                    for d in range(8):
                        nc.tensor.matmul(psv[:],
                                         wq[:, d * 384 + 256:d * 384 + 384],
                                         xt[:, d * CH:(d + 1) * CH],
                                         start=(d == 0), stop=(d == 7))
                    vtmp = wk.tile([128, CH], f32, tag="vt", bufs=2,
                                   name=f"vtmp{b}_{ch}")
                    nc.vector.tensor_scalar_add(vtmp[:], psv[:], bqc[:, 2:3])
                    psvt = ps.tile([128, CH], f32, tag="mm", bufs=2,
                                   name=f"psvt{b}_{ch}")
                    for sb in range(4):
                        nc.tensor.transpose(psvt[:, sb * 128:(sb + 1) * 128],
                                            vtmp[:, sb * 128:(sb + 1) * 128],
                                            idm[:])
                    for sb in range(4):
                        base = (ch * 4 + sb) * 130
                        nc.vector.tensor_copy(
                            vsb[b][:, base:base + 64],
                            psvt[:, sb * 128:sb * 128 + 64])
                        nc.vector.tensor_copy(
                            vsb[b][:, base + 65:base + 129],
                            psvt[:, sb * 128 + 64:sb * 128 + 128])

            def emit_attention(b):
                attn[b] = [wk.tile([64, T], bf16, tag=f"attn{h}", bufs=2,
                                   name=f"attn{h}_{b}") for h in range(HPC)]
                for qc in range(QC):
                    qs = qc * CH
                    nkv = 4 * (qc + 1)
                    pv = [ps.tile([65, CH], f32, tag=f"pv{h}", bufs=1,
                                  name=f"pv{h}_{b}_{qc}")
                          for h in range(HPC)]
                    def emit_scores(ki):
                        diag = ki - 4 * qc
                        c0 = 128 * diag if diag >= 0 else 0
                        st = ps.tile([128, 2 * CH], f32, tag="s", bufs=2,
                                     name=f"s_{b}_{qc}_{ki}")
                        st3 = st.rearrange("p (h c) -> p h c", h=2)
                        for h in range(HPC):
                            nc.tensor.matmul(
                                st3[:, h, c0:CH],
                                kt[b][64 * h:64 * h + 64,
                                      ki * 128:(ki + 1) * 128],
                                qt[b][64 * h:64 * h + 64,
                                      qs + c0:qs + CH],
                                start=True, stop=True,
                                tile_position=(64 * h, 0))
                        if diag >= 0:
                            ptx = ptd[diag]
                        else:
                            ptx = wk.tile([128, 2 * CH], bf16, tag="p",
                                          bufs=3, name=f"p_{b}_{qc}_{ki}")
                        pt3 = ptx.rearrange("p (h c) -> p h c", h=2)
                        nc.scalar.activation(pt3[:, :, c0:CH], st3[:, :, c0:CH],
                                             EXP, scale=SCALE)
                        if diag >= 0:
                            for h in range(HPC):
                                nc.vector.tensor_mul(pt3[:, h, c0:c0 + 128],
                                                     pt3[:, h, c0:c0 + 128],
                                                     htm[:])
                        return ptx

                    def emit_pv(ki, ptx):
                        for h in range(HPC):
                            vb = ki * 130 + 65 * h
                            nc.tensor.matmul(pv[h][0:65, :],
                                             vsb[b][:, vb:vb + 65],
                                             ptx[:, h * CH:(h + 1) * CH],
                                             start=(ki == 0),
                                             stop=(ki == nkv - 1))

                    for k0 in range(0, nkv, 2):
                        kis = [k for k in (k0, k0 + 1) if k < nkv]
                        pts = [emit_scores(k) for k in kis]
                        for k, ptx in zip(kis, pts):
                            emit_pv(k, ptx)
                    # unnormalized outputs + reciprocal of denominators
                    nc.vector.tensor_copy(attn[b][0][:, qs:qs + CH],
                                          pv[0][0:64, :])
                    nc.vector.tensor_copy(attn[b][1][:, qs:qs + CH],
                                          pv[1][0:64, :])
                    nc.vector.tensor_copy(den_s[64:65, qs:qs + CH],
                                          pv[0][64:65, :])
                    nc.vector.tensor_copy(den_s[64:65, T + qs:T + qs + CH],
                                          pv[1][64:65, :])
                # normalize: broadcast reciprocals across partitions via K=1
                # selector matmul, then one in-place multiply per q-chunk
                nc.scalar.activation(lnd[64:65, :], den_s[64:65, :],
                                     mybir.ActivationFunctionType.Ln)
                for qc in range(QC):
                    qs = qc * CH
                    for h in range(HPC):
                        bc = ps.tile([128, CH], f32, tag="mm", bufs=2,
                                     name=f"bc{b}_{qc}_{h}")
                        nc.tensor.matmul(bc[0:64, :], ones1[:],
                                         lnd[0:65, h * T + qs:h * T + qs + CH],
                                         start=True, stop=True)
                        rb = wk.tile([64, CH], bf16, tag="rb", bufs=2,
                                     name=f"rb{b}_{qc}_{h}")
                        nc.scalar.activation(rb[:], bc[0:64, :], EXP,
                                             scale=-1.0)
                        nc.vector.tensor_mul(attn[b][h][:, qs:qs + CH],
                                             attn[b][h][:, qs:qs + CH],
                                             rb[:])
                # re-shard head-sharded -> token-sharded
                a2a_in = dpool.tile([NCORES * 128, TPB], bf16,
                                    name=f"a2ai{b}")
                a2a_out[b] = dpool.tile([NCORES * 128, TPB], bf16,
                                        name=f"a2ao{b}")
                a2a_v = a2a_in.rearrange("(r hp p) c -> p r hp c",
                                         p=64, hp=2)
                for h in range(HPC):
                    nc.sync.dma_start(
                        a2a_v[:, :, h, :],
                        attn[b][h][:].rearrange("p (r c) -> p r c", c=TPB))
                if sim or no_collective:
                    nc.sync.dma_start(a2a_out[b][:], a2a_in[:])
                else:
                    nc.gpsimd.collective_compute(
                        "AllToAll", mybir.AluOpType.bypass,
                        replica_groups=[list(range(NCORES))],
                        ins=[a2a_in.opt()], outs=[a2a_out[b].opt()])

            def emit_fc(b):
                fci = wk.tile([128, 8 * TPB], bf16, tag="fci", bufs=2,
                              name=f"fci{b}")
                nc.sync.dma_start(
                    fci.rearrange("p (d c) -> p d c", d=8),
                    a2a_out[b].rearrange("(d p) c -> p d c", p=128))
                osts = wk.tile([128, 8 * TPB], f32, tag="ost", bufs=2,
                               name=f"ost{b}")
                for m in range(8):
                    pfc = ps.tile([128, CH], f32, tag="mm", bufs=2,
                                  name=f"pfc{b}_{m}")
                    for d in range(8):
                        nc.tensor.matmul(
                            pfc[:, 0:TPB],
                            wfc[:, d * D + m * 128:d * D + (m + 1) * 128],
                            fci[:, d * TPB:(d + 1) * TPB],
                            start=(d == 0), stop=(d == 7))
                    nc.vector.tensor_scalar_add(
                        osts[:, m * TPB:(m + 1) * TPB], pfc[:, 0:TPB],
                        bfcc[:, m:m + 1])
                nc.sync.dma_start(
                    outT[:, b * TPB:(b + 1) * TPB].rearrange(
                        "(m p) c -> p m c", p=128),
                    osts[:].rearrange("p (m c) -> p m c", c=TPB))

            emit_proj(0)
            for d in range(8):
                nc.sync.dma_start(wfc[:, d * D:(d + 1) * D],
                                  wfc_d[d * 128:(d + 1) * 128, :])
            if debug_out:
                nc.sync.dma_start(dq[:], qt[0][:].bitcast(f32))
                nc.sync.dma_start(dk[:], kt[0][:].bitcast(f32))
                nc.sync.dma_start(dv[:], vsb[0][:])
            for b in range(B):
                emit_attention(b)
                if debug_out and b == 0:
                    nc.sync.dma_start(da0[:], attn[0][0][:])
                    nc.sync.dma_start(da1[:], attn[0][1][:])
                    nc.sync.dma_start(drc[:], lnd[64:65, :])
                    nc.sync.dma_start(dao[:], a2a_out[0][:])
                if b + 1 < B:
                    emit_proj(b + 1)
                if b >= 1:
                    emit_fc(b - 1)
            emit_fc(B - 1)

    nc.compile()
    return nc


def _host_inputs(x, W_qkv, b_qkv, W_fc, b_fc):
    import ml_dtypes
    x = np.asarray(x, dtype=np.float32)
    W_qkv = np.asarray(W_qkv, dtype=np.float32)
    b_qkv = np.asarray(b_qkv, dtype=np.float32)
    W_fc = np.asarray(W_fc, dtype=np.float32)
    b_fc = np.asarray(b_fc, dtype=np.float32)

    xT = np.ascontiguousarray(x.reshape(BT, D).T).astype(ml_dtypes.bfloat16)
    wfc_b = np.ascontiguousarray(W_fc).astype(ml_dtypes.bfloat16)
    bfcc = np.ascontiguousarray(b_fc.reshape(8, 128).T)   # [128, 8]
    ht = (np.arange(128)[None, :] >= np.arange(128)[:, None]).astype(
        ml_dtypes.bfloat16)                               # keep iff c >= r
    ident = np.eye(128, dtype=np.float32)
    ones1 = np.zeros((65, 64), dtype=np.float32)
    ones1[64, :] = 1.0
    in_maps = []
    for c in range(NCORES):
        f0 = c * (HPC * HD)  # 128*c
        wq_c = np.ascontiguousarray(np.concatenate(
            [W_qkv[:, p * D + f0: p * D + f0 + 128] for p in range(3)],
            axis=1).astype(ml_dtypes.bfloat16))
        bq_c = np.ascontiguousarray(np.stack(
            [b_qkv[p * D + f0: p * D + f0 + 128] for p in range(3)],
            axis=1))                                      # [128, 3]
        in_maps.append({
            "xT": xT, "wqkv": wq_c, "wfc": wfc_b, "bqc": bq_c, "bfcc": bfcc,
            "ht": ht, "ident": ident, "ones1": ones1,
        })
    return in_maps


def _get_nc():
    if "nc" not in _CACHE:
        _CACHE["nc"] = _build()
    return _CACHE["nc"]


def _assemble(results):
    out = np.empty((BT, D), dtype=np.float32)
    for c in range(NCORES):
        o = results[c]["outT"]                  # [1024 feat, 1024 tok]
        for b in range(B):
            r0 = b * T + c * TPB
            out[r0:r0 + TPB, :] = o[:, b * TPB:(b + 1) * TPB].T
    return out.reshape(B, T, D)


def kernel(x, W_qkv, b_qkv, W_fc, b_fc):
    nc = _get_nc()
    in_maps = _host_inputs(x, W_qkv, b_qkv, W_fc, b_fc)
    res = run_bass_kernel_spmd(nc, in_maps, list(range(NCORES)))
    return _assemble(res.results)
